# revision 2
# baseline (speedup 1.0000x reference)
"""CLIF spiking-neuron recurrence kernel for 8 Trainium2 NeuronCores, v2.

Reference semantics (per element, T=64 sequential steps, gamma=0.5):
    u     = 0.5*u + x_t
    spike = (u >= 1.0)
    m     = s_prev * sigmoid(0.5*u) + spike
    s     = sigmoid(m)                       # carried (in-place sigmoid_)
    u     = u - spike*(1.0 + s)
Output: spikes [T, B, D] float32.

v2 design (vs v1's 3-group ACT/DVE/PE jam at 2.75us/step):
- The binding constraint is the per-step dependency cycle
  sigmoid(u) -> m-op -> sigmoid(m) -> reset-op -> sigmoid(u').
  v1 closed the loop through fp32 identity matmuls (2 passes + 2
  LDWEIGHTS ~1.1us inside the cycle). v2 applies the reset with a
  single custom DVE op (CLIF_VR) directly on PSUM - no PE in the cycle.
- G=2 groups of 256 cols; narrow per-group sigmoids keep the cycle
  short while ACT stays at 4 instructions/step.
- V = 2^t * u lives in PSUM (power-of-2 scaling exact in fp32); the
  leak folds into per-step constants. The x-add V += 2^(t+1)*x(t+1)
  runs in the shadow of the m-path, before the reset - legal because V
  is accumulate-only (x-early rounding order matched reference to
  1/33.5M elements in v1).
- Reset spike compare uses s >= c1, c1 = sigmoidLUT(1.0): m lies in
  (0, ~0.9] u [1, 2] (s_prev*sg <= sigmoid(2) < 0.9), so the compare
  only needs the LUT to separate 0.9 from 1.0.
- m-op spike compare keeps v1's (sg >= c), c = sigmoidLUT(0.5),
  HW-verified strictly monotone at 0.5.
- Output: sg streamed fp32; host applies spike = (sg >= c) with the
  on-device c, bit-identical to the DVE compares.
"""

import sys
import types

import numpy as np

try:
    import antenv.axon_hooks  # noqa: F401
except Exception:
    try:
        import antenv
        _hooks = types.ModuleType("antenv.axon_hooks")
        _hook_cell = [None]
        _hooks.set_axon_ntff_profile_hook = (
            lambda h: _hook_cell.__setitem__(0, h))
        _hooks.get_axon_ntff_profile_hook = lambda: _hook_cell[0]
        sys.modules["antenv.axon_hooks"] = _hooks
        antenv.axon_hooks = _hooks
    except Exception:
        pass

import concourse.bass as bass
import concourse.bacc as bacc
import concourse.mybir as mybir
import concourse.tile as tile
import concourse.dve_ops as dve_ops
from concourse.dve_spec import Spec, Src0, Src1, C0, C1, lower, _has_src1
from concourse.dve_uop import DveOpSpec
from concourse.bass_utils import run_bass_kernel_spmd

F32 = mybir.dt.float32
F32R = mybir.dt.float32r
AF = mybir.ActivationFunctionType
ALU = mybir.AluOpType

T = 64
B = 128
D = 4096
N_CORES = 8
P = 128
NPC = B * D // N_CORES          # 65536 elements per core
FDT = NPC // P                  # 512 free columns per core
W = FDT // 2                    # 256 per group
GROUPS = ((0, 0), (1, W))

# x-add engine: "pe" (fp32r identity matmul) or "dve" (tensor_tensor add)
XADD = "dve"
PREFETCH = 6

_NC_CACHE = None
LAST_RESULTS = None


def _register_dve_op(name, spec):
    for op in dve_ops.OPS:
        if op.name == name:
            return op
    shas = {}
    for ver in ("v3", "v4"):
        u = lower(spec, ver=ver)
        shas[ver] = DveOpSpec(name=name, opcode=1, uops=u,
                              rd1_en=_has_src1(spec)).sha(ver)
    op = dve_ops.DveOp(name, spec, subdim=False, uops_sha=shas)
    dve_ops.OPS.append(op)
    dve_ops._SUB_OPCODE_FOR_NAME[name] = (
        dve_ops._CUSTOM_DVE_ROW_BASE + len(dve_ops.OPS) - 1)
    dve_ops.CUSTOM_DVE_SPECS[name] = spec
    return op


# m = s_prev*sg + (sg >= c)            in0=s_prev, in1=sg, s0=c
CLIF_M = _register_dve_op("CLIF_M_ANT", Spec(
    body=Src0 * Src1 + (Src1 >= C0),
    reference=lambda in0, in1, s0, s1, imm2:
        in0 * in1 + (in1 >= s0).astype(np.float32),
))
# V' = V + (s >= c1)*(s*zneg + zneg)   in0=V, in1=s, s0=c1, s1=zneg=-2^t
CLIF_VR = _register_dve_op("CLIF_VR_ANT", Spec(
    body=Src0 + (Src1 >= C0) * (Src1 * C1 + C1),
    reference=lambda in0, in1, s0, s1, imm2:
        in0 + (in1 >= s0).astype(np.float32) * (in1 * s1 + s1),
))


def _build():
    nc = bacc.Bacc(None, target_bir_lowering=False, debug=False,
                   num_devices=N_CORES)

    xdt = F32
    xs = nc.declare_dram_parameter("xs", [T, P, FDT], xdt, isOutput=False)
    out = nc.declare_dram_parameter("out", [T, P, FDT], F32, isOutput=True)
    cout = nc.declare_dram_parameter("cout", [P, 1], F32, isOutput=True)
    if XADD == "pe":
        wt = nc.declare_dram_parameter("wt", [P, P], F32, isOutput=False)

    with tile.TileContext(nc) as tc:
        with (
            tc.tile_pool(name="cpool", bufs=1) as cpool,
            tc.tile_pool(name="xpool", bufs=PREFETCH + 2) as xpool,
            tc.tile_pool(name="sgpool", bufs=6) as sgpool,
            tc.tile_pool(name="spool", bufs=1) as spool,
            tc.tile_pool(name="mpool", bufs=1) as mpool,
            tc.tile_pool(name="vpool", bufs=1, space="PSUM") as vpool,
        ):
            # --- one-time constants ---------------------------------------
            halft = cpool.tile([P, 1], F32, tag="half")
            nc.gpsimd.memset(halft[:], 0.5)
            onet = cpool.tile([P, 1], F32, tag="one")
            nc.gpsimd.memset(onet[:], 1.0)
            ct = cpool.tile([P, 1], F32, tag="c")
            nc.scalar.activation(ct[:], halft[:], AF.Sigmoid, bias=0.0, scale=1.0)
            c1t = cpool.tile([P, 1], F32, tag="c1")
            nc.scalar.activation(c1t[:], onet[:], AF.Sigmoid, bias=0.0, scale=1.0)
            c_ap = ct[:, 0:1]
            c1_ap = c1t[:, 0:1]

            if XADD == "pe":
                eye = cpool.tile([P, P], F32, tag="eye")
                nc.sync.dma_start(eye[:], wt[:])

            # --- state ----------------------------------------------------
            s_t = {}
            m_t = {}
            V = {}
            for g, o in GROUPS:
                s0 = spool.tile([P, W], F32, tag=f"s{g}")
                nc.gpsimd.memset(s0[:], 0.0)
                s_t[g] = s0
                mt_ = mpool.tile([P, W], F32, tag=f"m{g}")
                m_t[g] = mt_
                vt_ = vpool.tile([P, W], F32, tag=f"V{g}")
                V[g] = vt_

            # x prefetch ring; xtiles[t] holds 2^t * x(t)
            xtiles = {}
            for i in range(min(PREFETCH, T)):
                xt_ = xpool.tile([P, FDT], xdt, tag="x")
                nc.sync.dma_start(xt_[:], xs[i])
                xtiles[i] = xt_

            # V(0) = 2^0 * u(0) = x(0)
            for g, o in GROUPS:
                nc.vector.tensor_copy(V[g][:], xtiles[0][:, o:o + W])

            if XADD == "pe":
                # PE clock warm-up + per-step fillers keep the HAM clock
                # gate at 2.4 GHz so the x-add matmuls stay off the cycle
                junkv = vpool.tile([P, 128], F32, tag="junkv")
                for _ in range(12):
                    nc.tensor.matmul(junkv[:], eye[:], eye[:],
                                     start=True, stop=True,
                                     skip_group_check=True)

            # --- the recurrence -------------------------------------------
            # per-step queue order encodes the pipelined schedule:
            #   ACT:  sg_L, sg_R, s_L, s_R      (narrow: cycle-bound regime)
            #   DVE:  A_L, A_R, [xadds], VR_L, VR_R
            #   PE:   xadd_L, xadd_R  (fp32r identity accumulate, off-cycle)
            for t in range(T):
                sc = float(2.0 ** (-t - 1))
                zneg = float(-(2.0 ** t))
                last = (t == T - 1)

                if t + PREFETCH < T:
                    xt_ = xpool.tile([P, FDT], xdt, tag="x")
                    nc.sync.dma_start(xt_[:], xs[t + PREFETCH])
                    xtiles[t + PREFETCH] = xt_

                sgw = sgpool.tile([P, FDT], F32, tag="sg")
                for g, o in GROUPS:
                    # sg_g(t) = sigmoid(2^-(t+1) * V_g)
                    nc.scalar.activation(sgw[:, o:o + W], V[g][:], AF.Sigmoid,
                                         bias=0.0, scale=sc)
                    nc.sync.dma_start(out[t][:, o:o + W], sgw[:, o:o + W])
                    if not last:
                        # m_g = s_g*sg_g + (sg_g >= c)   (gates sigma_s: first
                        # on the DVE queue)
                        nc.vector._custom_dve(CLIF_M, out=m_t[g][:],
                                              in0=s_t[g][:],
                                              in1=sgw[:, o:o + W], s0=c_ap)

                if not last:
                    xn = xtiles.pop(t + 1)
                    for g, o in GROUPS:
                        # V_g += 2^(t+1)*x_g(t+1): off the critical cycle,
                        # pre-reset accumulation order (v1-sanctioned)
                        nc.vector.tensor_tensor(V[g][:], V[g][:],
                                                xn[:, o:o + W], ALU.add)
                    if XADD == "pe":
                        # dependency-free filler keeps the PE clock warm
                        nc.tensor.matmul(junkv[:], eye[:], eye[:],
                                         start=True, stop=True,
                                         skip_group_check=True)

                    for g, o in GROUPS:
                        # s_g = sigmoid(m_g)
                        nc.scalar.activation(s_t[g][:], m_t[g][:], AF.Sigmoid,
                                             bias=0.0, scale=1.0)
                        # V_g += (s_g >= c1) * (-2^t) * (1 + s_g)
                        nc.vector._custom_dve(CLIF_VR, out=V[g][:],
                                              in0=V[g][:], in1=s_t[g][:],
                                              s0=c1_ap, s1=zneg)

                xtiles.pop(t, None)

            nc.sync.dma_start(cout[:], ct[:])

    nc.compile()
    return nc


def _get_nc():
    global _NC_CACHE
    if _NC_CACHE is None:
        _NC_CACHE = _build()
    return _NC_CACHE


def kernel(x_seq: np.ndarray) -> np.ndarray:
    global LAST_RESULTS
    x = np.ascontiguousarray(x_seq, dtype=np.float32)
    assert x.shape == (T, B, D), x.shape

    # xs[t] = 2^t * x(t): exact in fp32; consumed as V(0)=xs[0] and
    # V += xs[t+1] at step t.
    scale = (2.0 ** np.arange(T, dtype=np.float64)).astype(np.float32)
    xsc = x.reshape(T, -1) * scale[:, None]
    xsc = xsc.reshape(T, N_CORES, P, FDT)

    nc = _get_nc()
    in_maps = []
    for c in range(N_CORES):
        m = {"xs": np.ascontiguousarray(xsc[:, c])}
        if XADD == "pe":
            m["wt"] = np.eye(P, dtype=np.float32)
        in_maps.append(m)
    LAST_RESULTS = run_bass_kernel_spmd(nc, in_maps, list(range(N_CORES)))

    full = np.empty((T, N_CORES, P, FDT), dtype=np.float32)
    for c in range(N_CORES):
        res = LAST_RESULTS.results[c]
        c_val = np.asarray(res["cout"], dtype=np.float32)[0, 0]
        sg = np.asarray(res["out"], dtype=np.float32)
        full[:, c] = (sg >= c_val).astype(np.float32)
    return full.reshape(T, B, D)


# revision 3
# speedup vs baseline: 1.0032x; 1.0032x over previous
"""CLIF spiking-neuron recurrence kernel for 8 Trainium2 NeuronCores, v2.

Reference semantics (per element, T=64 sequential steps, gamma=0.5):
    u     = 0.5*u + x_t
    spike = (u >= 1.0)
    m     = s_prev * sigmoid(0.5*u) + spike
    s     = sigmoid(m)                       # carried (in-place sigmoid_)
    u     = u - spike*(1.0 + s)
Output: spikes [T, B, D] float32.

v2 design (vs v1's 3-group ACT/DVE/PE jam at 2.75us/step):
- The binding constraint is the per-step dependency cycle
  sigmoid(u) -> m-op -> sigmoid(m) -> reset-op -> sigmoid(u').
  v1 closed the loop through fp32 identity matmuls (2 passes + 2
  LDWEIGHTS ~1.1us inside the cycle). v2 applies the reset with a
  single custom DVE op (CLIF_VR) directly on PSUM - no PE in the cycle.
- G=2 groups of 256 cols; narrow per-group sigmoids keep the cycle
  short while ACT stays at 4 instructions/step.
- V = 2^t * u lives in PSUM (power-of-2 scaling exact in fp32); the
  leak folds into per-step constants. The x-add V += 2^(t+1)*x(t+1)
  runs in the shadow of the m-path, before the reset - legal because V
  is accumulate-only (x-early rounding order matched reference to
  1/33.5M elements in v1).
- Reset spike compare uses s >= c1, c1 = sigmoidLUT(1.0): m lies in
  (0, ~0.9] u [1, 2] (s_prev*sg <= sigmoid(2) < 0.9), so the compare
  only needs the LUT to separate 0.9 from 1.0.
- m-op spike compare keeps v1's (sg >= c), c = sigmoidLUT(0.5),
  HW-verified strictly monotone at 0.5.
- Output: sg streamed fp32; host applies spike = (sg >= c) with the
  on-device c, bit-identical to the DVE compares.
"""

import sys
import types

import numpy as np

try:
    import antenv.axon_hooks  # noqa: F401
except Exception:
    try:
        import antenv
        _hooks = types.ModuleType("antenv.axon_hooks")
        _hook_cell = [None]
        _hooks.set_axon_ntff_profile_hook = (
            lambda h: _hook_cell.__setitem__(0, h))
        _hooks.get_axon_ntff_profile_hook = lambda: _hook_cell[0]
        sys.modules["antenv.axon_hooks"] = _hooks
        antenv.axon_hooks = _hooks
    except Exception:
        pass

import concourse.bass as bass
import concourse.bacc as bacc
import concourse.mybir as mybir
import concourse.tile as tile
import concourse.dve_ops as dve_ops
from concourse.dve_spec import Spec, Src0, Src1, C0, C1, lower, _has_src1
from concourse.dve_uop import DveOpSpec
from concourse.bass_utils import run_bass_kernel_spmd

F32 = mybir.dt.float32
F32R = mybir.dt.float32r
AF = mybir.ActivationFunctionType
ALU = mybir.AluOpType

T = 64
B = 128
D = 4096
N_CORES = 8
P = 128
NPC = B * D // N_CORES          # 65536 elements per core
FDT = NPC // P                  # 512 free columns per core
W = FDT // 2                    # 256 per group
GROUPS = ((0, 0), (1, W))

# x-add engine: "dve" (tensor_tensor add into PSUM). "pe" (fp32 identity
# matmul) was measured worse: fp32 matmuls run 2 half-speed passes with
# LDWEIGHTS pairs and the PE idles into its low p-state; fp32r is inexact.
XADD = "dve"
PREFETCH = 6

_NC_CACHE = None
LAST_RESULTS = None


def _register_dve_op(name, spec):
    for op in dve_ops.OPS:
        if op.name == name:
            return op
    shas = {}
    for ver in ("v3", "v4"):
        u = lower(spec, ver=ver)
        shas[ver] = DveOpSpec(name=name, opcode=1, uops=u,
                              rd1_en=_has_src1(spec)).sha(ver)
    op = dve_ops.DveOp(name, spec, subdim=False, uops_sha=shas)
    dve_ops.OPS.append(op)
    dve_ops._SUB_OPCODE_FOR_NAME[name] = (
        dve_ops._CUSTOM_DVE_ROW_BASE + len(dve_ops.OPS) - 1)
    dve_ops.CUSTOM_DVE_SPECS[name] = spec
    return op


# m = s_prev*sg + (sg >= c)            in0=s_prev, in1=sg, s0=c
CLIF_M = _register_dve_op("CLIF_M_ANT", Spec(
    body=Src0 * Src1 + (Src1 >= C0),
    reference=lambda in0, in1, s0, s1, imm2:
        in0 * in1 + (in1 >= s0).astype(np.float32),
))
# V' = V + (s >= c1)*(s*zneg + zneg)   in0=V, in1=s, s0=c1, s1=zneg=-2^t
CLIF_VR = _register_dve_op("CLIF_VR_ANT", Spec(
    body=Src0 + (Src1 >= C0) * (Src1 * C1 + C1),
    reference=lambda in0, in1, s0, s1, imm2:
        in0 + (in1 >= s0).astype(np.float32) * (in1 * s1 + s1),
))


def _build():
    nc = bacc.Bacc(None, target_bir_lowering=False, debug=False,
                   num_devices=N_CORES)

    xdt = F32
    xs = nc.declare_dram_parameter("xs", [T, P, FDT], xdt, isOutput=False)
    out = nc.declare_dram_parameter("out", [T, P, FDT], F32, isOutput=True)
    cout = nc.declare_dram_parameter("cout", [P, 1], F32, isOutput=True)
    if XADD == "pe":
        wt = nc.declare_dram_parameter("wt", [P, P], F32, isOutput=False)

    with tile.TileContext(nc) as tc:
        with (
            tc.tile_pool(name="cpool", bufs=1) as cpool,
            tc.tile_pool(name="xpool", bufs=PREFETCH + 2) as xpool,
            tc.tile_pool(name="sgpool", bufs=6) as sgpool,
            tc.tile_pool(name="spool", bufs=1) as spool,
            tc.tile_pool(name="mpool", bufs=1) as mpool,
            tc.tile_pool(name="vpool", bufs=1, space="PSUM") as vpool,
        ):
            # --- one-time constants ---------------------------------------
            halft = cpool.tile([P, 1], F32, tag="half")
            nc.gpsimd.memset(halft[:], 0.5)
            onet = cpool.tile([P, 1], F32, tag="one")
            nc.gpsimd.memset(onet[:], 1.0)
            ct = cpool.tile([P, 1], F32, tag="c")
            nc.scalar.activation(ct[:], halft[:], AF.Sigmoid, bias=0.0, scale=1.0)
            c1t = cpool.tile([P, 1], F32, tag="c1")
            nc.scalar.activation(c1t[:], onet[:], AF.Sigmoid, bias=0.0, scale=1.0)
            c_ap = ct[:, 0:1]
            c1_ap = c1t[:, 0:1]

            if XADD == "pe":
                eye = cpool.tile([P, P], F32, tag="eye")
                nc.sync.dma_start(eye[:], wt[:])

            # --- state ----------------------------------------------------
            s_t = {}
            m_t = {}
            V = {}
            for g, o in GROUPS:
                s0 = spool.tile([P, W], F32, tag=f"s{g}")
                nc.gpsimd.memset(s0[:], 0.0)
                s_t[g] = s0
                mt_ = mpool.tile([P, W], F32, tag=f"m{g}")
                m_t[g] = mt_
                vt_ = vpool.tile([P, W], F32, tag=f"V{g}")
                V[g] = vt_

            # x prefetch ring; xtiles[t] holds 2^t * x(t)
            xtiles = {}
            for i in range(min(PREFETCH, T)):
                xt_ = xpool.tile([P, FDT], xdt, tag="x")
                nc.sync.dma_start(xt_[:], xs[i])
                xtiles[i] = xt_

            # V(0) = 2^0 * u(0) = x(0)
            for g, o in GROUPS:
                nc.vector.tensor_copy(V[g][:], xtiles[0][:, o:o + W])

            if XADD == "pe":
                # PE clock warm-up + per-step fillers keep the HAM clock
                # gate at 2.4 GHz so the x-add matmuls stay off the cycle
                junkv = vpool.tile([P, 128], F32, tag="junkv")
                for _ in range(12):
                    nc.tensor.matmul(junkv[:], eye[:], eye[:],
                                     start=True, stop=True,
                                     skip_group_check=True)

            # --- the recurrence -------------------------------------------
            # per-step queue order encodes the pipelined schedule:
            #   ACT:  sg_L, sg_R, s_L, s_R      (narrow: cycle-bound regime)
            #   DVE:  A_L, A_R, xadd_L, xadd_R, VR_L, VR_R
            # Separate per-group V tiles and narrow per-group ops are
            # deliberate: a shared V arena or widened ops serialize the
            # two groups' pipelines (measured 235-245us vs 173us).
            for t in range(T):
                sc = float(2.0 ** (-t - 1))
                zneg = float(-(2.0 ** t))
                last = (t == T - 1)

                if t + PREFETCH < T:
                    xt_ = xpool.tile([P, FDT], xdt, tag="x")
                    nc.sync.dma_start(xt_[:], xs[t + PREFETCH])
                    xtiles[t + PREFETCH] = xt_

                sgw = sgpool.tile([P, FDT], F32, tag="sg")
                for g, o in GROUPS:
                    # sg_g(t) = sigmoid(2^-(t+1) * V_g)
                    nc.scalar.activation(sgw[:, o:o + W], V[g][:], AF.Sigmoid,
                                         bias=0.0, scale=sc)
                    nc.sync.dma_start(out[t][:, o:o + W], sgw[:, o:o + W])
                    if not last:
                        # m_g = s_g*sg_g + (sg_g >= c)   (gates sigma_s: first
                        # on the DVE queue)
                        nc.vector._custom_dve(CLIF_M, out=m_t[g][:],
                                              in0=s_t[g][:],
                                              in1=sgw[:, o:o + W], s0=c_ap)

                if not last:
                    xn = xtiles.pop(t + 1)
                    for g, o in GROUPS:
                        # V_g += 2^(t+1)*x_g(t+1): off the critical cycle,
                        # pre-reset accumulation order (v1-sanctioned)
                        nc.vector.tensor_tensor(V[g][:], V[g][:],
                                                xn[:, o:o + W], ALU.add)
                    if XADD == "pe":
                        # dependency-free filler keeps the PE clock warm
                        nc.tensor.matmul(junkv[:], eye[:], eye[:],
                                         start=True, stop=True,
                                         skip_group_check=True)

                    for g, o in GROUPS:
                        # s_g = sigmoid(m_g)
                        nc.scalar.activation(s_t[g][:], m_t[g][:], AF.Sigmoid,
                                             bias=0.0, scale=1.0)
                        # V_g += (s_g >= c1) * (-2^t) * (1 + s_g)
                        nc.vector._custom_dve(CLIF_VR, out=V[g][:],
                                              in0=V[g][:], in1=s_t[g][:],
                                              s0=c1_ap, s1=zneg)

                xtiles.pop(t, None)

            nc.sync.dma_start(cout[:], ct[:])

    nc.compile()
    return nc


def _get_nc():
    global _NC_CACHE
    if _NC_CACHE is None:
        _NC_CACHE = _build()
    return _NC_CACHE


def kernel(x_seq: np.ndarray) -> np.ndarray:
    global LAST_RESULTS
    x = np.ascontiguousarray(x_seq, dtype=np.float32)
    assert x.shape == (T, B, D), x.shape

    # xs[t] = 2^t * x(t): exact in fp32; consumed as V(0)=xs[0] and
    # V += xs[t+1] at step t.
    scale = (2.0 ** np.arange(T, dtype=np.float64)).astype(np.float32)
    xsc = x.reshape(T, -1) * scale[:, None]
    xsc = xsc.reshape(T, N_CORES, P, FDT)

    nc = _get_nc()
    in_maps = []
    for c in range(N_CORES):
        m = {"xs": np.ascontiguousarray(xsc[:, c])}
        if XADD == "pe":
            m["wt"] = np.eye(P, dtype=np.float32)
        in_maps.append(m)
    LAST_RESULTS = run_bass_kernel_spmd(nc, in_maps, list(range(N_CORES)))

    full = np.empty((T, N_CORES, P, FDT), dtype=np.float32)
    for c in range(N_CORES):
        res = LAST_RESULTS.results[c]
        c_val = np.asarray(res["cout"], dtype=np.float32)[0, 0]
        sg = np.asarray(res["out"], dtype=np.float32)
        full[:, c] = (sg >= c_val).astype(np.float32)
    return full.reshape(T, B, D)


# revision 4
# speedup vs baseline: 1.0069x; 1.0037x over previous
"""CLIF spiking-neuron recurrence kernel for 8 Trainium2 NeuronCores, v2.

Reference semantics (per element, T=64 sequential steps, gamma=0.5):
    u     = 0.5*u + x_t
    spike = (u >= 1.0)
    m     = s_prev * sigmoid(0.5*u) + spike
    s     = sigmoid(m)                       # carried (in-place sigmoid_)
    u     = u - spike*(1.0 + s)
Output: spikes [T, B, D] float32.

v2 design (vs v1's 3-group ACT/DVE/PE jam at 2.75us/step):
- The binding constraint is the per-step dependency cycle
  sigmoid(u) -> m-op -> sigmoid(m) -> reset-op -> sigmoid(u').
  v1 closed the loop through fp32 identity matmuls (2 passes + 2
  LDWEIGHTS ~1.1us inside the cycle). v2 applies the reset with a
  single custom DVE op (CLIF_VR) directly on PSUM - no PE in the cycle.
- G=2 groups of 256 cols; narrow per-group sigmoids keep the cycle
  short while ACT stays at 4 instructions/step.
- V = 2^t * u lives in PSUM (power-of-2 scaling exact in fp32); the
  leak folds into per-step constants. The x-add V += 2^(t+1)*x(t+1)
  runs in the shadow of the m-path, before the reset - legal because V
  is accumulate-only (x-early rounding order matched reference to
  1/33.5M elements in v1).
- Reset spike compare uses s >= c1, c1 = sigmoidLUT(1.0): m lies in
  (0, ~0.9] u [1, 2] (s_prev*sg <= sigmoid(2) < 0.9), so the compare
  only needs the LUT to separate 0.9 from 1.0.
- m-op spike compare keeps v1's (sg >= c), c = sigmoidLUT(0.5),
  HW-verified strictly monotone at 0.5.
- Output: sg streamed fp32; host applies spike = (sg >= c) with the
  on-device c, bit-identical to the DVE compares.
"""

import sys
import types

import numpy as np

try:
    import antenv.axon_hooks  # noqa: F401
except Exception:
    try:
        import antenv
        _hooks = types.ModuleType("antenv.axon_hooks")
        _hook_cell = [None]
        _hooks.set_axon_ntff_profile_hook = (
            lambda h: _hook_cell.__setitem__(0, h))
        _hooks.get_axon_ntff_profile_hook = lambda: _hook_cell[0]
        sys.modules["antenv.axon_hooks"] = _hooks
        antenv.axon_hooks = _hooks
    except Exception:
        pass

import concourse.bass as bass
import concourse.bacc as bacc
import concourse.mybir as mybir
import concourse.tile as tile
import concourse.dve_ops as dve_ops
from concourse.dve_spec import Spec, Src0, Src1, C0, C1, lower, _has_src1
from concourse.dve_uop import DveOpSpec
from concourse.bass_utils import run_bass_kernel_spmd

F32 = mybir.dt.float32
F32R = mybir.dt.float32r
AF = mybir.ActivationFunctionType
ALU = mybir.AluOpType

T = 64
B = 128
D = 4096
N_CORES = 8
P = 128
NPC = B * D // N_CORES          # 65536 elements per core
FDT = NPC // P                  # 512 free columns per core
W = FDT // 2                    # 256 per group
GROUPS = ((0, 0), (1, W))

# x-add engine: "pe" (fp32r identity matmul) or "dve" (tensor_tensor add)
XADD = "dve"
PREFETCH = 8

_NC_CACHE = None
LAST_RESULTS = None


def _register_dve_op(name, spec):
    for op in dve_ops.OPS:
        if op.name == name:
            return op
    shas = {}
    for ver in ("v3", "v4"):
        u = lower(spec, ver=ver)
        shas[ver] = DveOpSpec(name=name, opcode=1, uops=u,
                              rd1_en=_has_src1(spec)).sha(ver)
    op = dve_ops.DveOp(name, spec, subdim=False, uops_sha=shas)
    dve_ops.OPS.append(op)
    dve_ops._SUB_OPCODE_FOR_NAME[name] = (
        dve_ops._CUSTOM_DVE_ROW_BASE + len(dve_ops.OPS) - 1)
    dve_ops.CUSTOM_DVE_SPECS[name] = spec
    return op


# m = s_prev*sg + (sg >= c)            in0=s_prev, in1=sg, s0=c
CLIF_M = _register_dve_op("CLIF_M_ANT", Spec(
    body=Src0 * Src1 + (Src1 >= C0),
    reference=lambda in0, in1, s0, s1, imm2:
        in0 * in1 + (in1 >= s0).astype(np.float32),
))
# V' = V + (s >= c1)*(s*zneg + zneg)   in0=V, in1=s, s0=c1, s1=zneg=-2^t
CLIF_VR = _register_dve_op("CLIF_VR_ANT", Spec(
    body=Src0 + (Src1 >= C0) * (Src1 * C1 + C1),
    reference=lambda in0, in1, s0, s1, imm2:
        in0 + (in1 >= s0).astype(np.float32) * (in1 * s1 + s1),
))


def _build():
    nc = bacc.Bacc(None, target_bir_lowering=False, debug=False,
                   num_devices=N_CORES)

    xdt = F32
    xs = nc.declare_dram_parameter("xs", [T, P, FDT], xdt, isOutput=False)
    out = nc.declare_dram_parameter("out", [T, P, FDT], F32, isOutput=True)
    cout = nc.declare_dram_parameter("cout", [P, 1], F32, isOutput=True)
    if XADD == "pe":
        wt = nc.declare_dram_parameter("wt", [P, P], F32, isOutput=False)

    with tile.TileContext(nc) as tc:
        with (
            tc.tile_pool(name="cpool", bufs=1) as cpool,
            tc.tile_pool(name="xpool", bufs=PREFETCH + 2) as xpool,
            tc.tile_pool(name="sgpool", bufs=6) as sgpool,
            tc.tile_pool(name="spool", bufs=1) as spool,
            tc.tile_pool(name="mpool", bufs=1) as mpool,
            tc.tile_pool(name="vpool", bufs=1, space="PSUM") as vpool,
        ):
            # --- one-time constants ---------------------------------------
            halft = cpool.tile([P, 1], F32, tag="half")
            nc.gpsimd.memset(halft[:], 0.5)
            onet = cpool.tile([P, 1], F32, tag="one")
            nc.gpsimd.memset(onet[:], 1.0)
            ct = cpool.tile([P, 1], F32, tag="c")
            nc.scalar.activation(ct[:], halft[:], AF.Sigmoid, bias=0.0, scale=1.0)
            c1t = cpool.tile([P, 1], F32, tag="c1")
            nc.scalar.activation(c1t[:], onet[:], AF.Sigmoid, bias=0.0, scale=1.0)
            c_ap = ct[:, 0:1]
            c1_ap = c1t[:, 0:1]

            if XADD == "pe":
                eye = cpool.tile([P, P], F32, tag="eye")
                nc.sync.dma_start(eye[:], wt[:])

            # --- state ----------------------------------------------------
            s_t = {}
            m_t = {}
            V = {}
            for g, o in GROUPS:
                s0 = spool.tile([P, W], F32, tag=f"s{g}")
                nc.vector.memset(s0[:], 0.0)
                s_t[g] = s0
                mt_ = mpool.tile([P, W], F32, tag=f"m{g}")
                m_t[g] = mt_
                vt_ = vpool.tile([P, W], F32, tag=f"V{g}")
                V[g] = vt_
                # pad to the next 2KB PSUM bank: concurrent ACT-read of V_L
                # and DVE-write of V_R must not share a bank
                vpad = vpool.tile([P, W], F32, tag=f"Vpad{g}")

            # x prefetch ring; xtiles[t] holds 2^t * x(t)
            xtiles = {}
            for i in range(min(PREFETCH, T)):
                xt_ = xpool.tile([P, FDT], xdt, tag="x")
                nc.sync.dma_start(xt_[:], xs[i])
                xtiles[i] = xt_

            # V(0) = 2^0 * u(0) = x(0)
            for g, o in GROUPS:
                nc.vector.tensor_copy(V[g][:], xtiles[0][:, o:o + W])

            if XADD == "pe":
                # PE clock warm-up + per-step fillers keep the HAM clock
                # gate at 2.4 GHz so the x-add matmuls stay off the cycle
                junkv = vpool.tile([P, 128], F32, tag="junkv")
                for _ in range(12):
                    nc.tensor.matmul(junkv[:], eye[:], eye[:],
                                     start=True, stop=True,
                                     skip_group_check=True)

            # --- the recurrence -------------------------------------------
            # per-step queue order encodes the pipelined schedule:
            #   ACT:  sg_L, sg_R, s_L, s_R      (narrow: cycle-bound regime)
            #   DVE:  A_L, A_R, [xadds], VR_L, VR_R
            #   PE:   xadd_L, xadd_R  (fp32r identity accumulate, off-cycle)
            for t in range(T):
                sc = float(2.0 ** (-t - 1))
                zneg = float(-(2.0 ** t))
                last = (t == T - 1)

                if t + PREFETCH < T:
                    xt_ = xpool.tile([P, FDT], xdt, tag="x")
                    nc.sync.dma_start(xt_[:], xs[t + PREFETCH])
                    xtiles[t + PREFETCH] = xt_

                sgw = sgpool.tile([P, FDT], F32, tag="sg")
                for g, o in GROUPS:
                    # sg_g(t) = sigmoid(2^-(t+1) * V_g); at t=0 V == x(0), so
                    # read the x tile directly - the first sigmoid starts as
                    # soon as the x(0) DMA lands, parallel to the V-init copy
                    sg_src = xtiles[0][:, o:o + W] if t == 0 else V[g][:]
                    nc.scalar.activation(sgw[:, o:o + W], sg_src, AF.Sigmoid,
                                         bias=0.0, scale=sc)
                    nc.sync.dma_start(out[t][:, o:o + W], sgw[:, o:o + W])
                    if not last:
                        # m_g = s_g*sg_g + (sg_g >= c)   (gates sigma_s: first
                        # on the DVE queue)
                        nc.vector._custom_dve(CLIF_M, out=m_t[g][:],
                                              in0=s_t[g][:],
                                              in1=sgw[:, o:o + W], s0=c_ap)

                if not last:
                    xn = xtiles.pop(t + 1)
                    for g, o in GROUPS:
                        # V_g += 2^(t+1)*x_g(t+1): off the critical cycle,
                        # pre-reset accumulation order (v1-sanctioned)
                        nc.vector.tensor_tensor(V[g][:], V[g][:],
                                                xn[:, o:o + W], ALU.add)
                    if XADD == "pe":
                        # dependency-free filler keeps the PE clock warm
                        nc.tensor.matmul(junkv[:], eye[:], eye[:],
                                         start=True, stop=True,
                                         skip_group_check=True)

                    for g, o in GROUPS:
                        # s_g = sigmoid(m_g)
                        nc.scalar.activation(s_t[g][:], m_t[g][:], AF.Sigmoid,
                                             bias=0.0, scale=1.0)
                        # V_g += (s_g >= c1) * (-2^t) * (1 + s_g)
                        nc.vector._custom_dve(CLIF_VR, out=V[g][:],
                                              in0=V[g][:], in1=s_t[g][:],
                                              s0=c1_ap, s1=zneg)

                xtiles.pop(t, None)

            nc.sync.dma_start(cout[:], ct[:])

    nc.compile()
    return nc


def _get_nc():
    global _NC_CACHE
    if _NC_CACHE is None:
        _NC_CACHE = _build()
    return _NC_CACHE


def kernel(x_seq: np.ndarray) -> np.ndarray:
    global LAST_RESULTS
    x = np.ascontiguousarray(x_seq, dtype=np.float32)
    assert x.shape == (T, B, D), x.shape

    # xs[t] = 2^t * x(t): exact in fp32; consumed as V(0)=xs[0] and
    # V += xs[t+1] at step t.
    scale = (2.0 ** np.arange(T, dtype=np.float64)).astype(np.float32)
    xsc = x.reshape(T, -1) * scale[:, None]
    xsc = xsc.reshape(T, N_CORES, P, FDT)

    nc = _get_nc()
    in_maps = []
    for c in range(N_CORES):
        m = {"xs": np.ascontiguousarray(xsc[:, c])}
        if XADD == "pe":
            m["wt"] = np.eye(P, dtype=np.float32)
        in_maps.append(m)
    LAST_RESULTS = run_bass_kernel_spmd(nc, in_maps, list(range(N_CORES)))

    full = np.empty((T, N_CORES, P, FDT), dtype=np.float32)
    for c in range(N_CORES):
        res = LAST_RESULTS.results[c]
        c_val = np.asarray(res["cout"], dtype=np.float32)[0, 0]
        sg = np.asarray(res["out"], dtype=np.float32)
        full[:, c] = (sg >= c_val).astype(np.float32)
    return full.reshape(T, B, D)
